# revision 16
# baseline (speedup 1.0000x reference)
"""Trainium2 Bass kernel for a GPT-2 style transformer block
(S=3072, E=1024, 16 heads, MLP 4x), distributed over 8 NeuronCores.

Sharding (unchanged from v1):
  - LN1 sequence-parallel, AllGather of normalized+transposed activations
    (bf16), attention tensor-parallel over heads (2/core), per-head AllToAll
    back to sequence-parallel, then proj+LN2+MLP per seq chunk.

v2 performance restructure:
  - LN affine weights folded into the following matmul weights on the host
    (exact), so LN emits plain normalized values and the per-block
    scale/bias DVE ops disappear.
  - bf16 everywhere on device (activations, scores, attention probs, all
    collective payloads); fp32 only in PSUM accumulation, LN stats and
    residuals.
  - Attention processes 512-wide query chunks; scores for two consecutive
    key blocks land in one 2-bank PSUM tile and are exponentiated by a
    single ACT instruction (the 352-cycle ACT fixed cost is the attention
    bottleneck).  Causal masking of diagonal blocks stays post-exp on
    GPSIMD (affine_select), one call per block pair.
  - The external mask enters as the per-partition ACT bias only when the
    mask actually masks something (separate cached program variant);
    biases similarly get matmul-group ones-outer-product adds only when
    nonzero.
  - Head-outer attention loop so head 0's AllToAll overlaps head 1's
    compute.
  - Batched PSUM evacuations (4 transposed 128-blocks per copy), engine
    alternation between ACT and DVE for all PSUM->SBUF moves.
  - MLP weights streamed with deep buffering; per-core HBM traffic is
    ~20 MB against ~85 us of compute, so the stream hides completely.
"""

import numpy as np

E, H, I = 1024, 16, 4096
W = 8
MASK = -10000.0
QC = 512  # query-chunk width (one PSUM bank of fp32)

_CACHE = {}


def _build(SS: int, dt_name: str, masked: bool = False,
           biases=(False, False, False, False), mock_cc: bool = False):
    """Build the SPMD Bass program.
    dt_name: 'bf16' (fast, HW) or 'float32' (attention in fp32, for sim).
    masked: external mask has False entries -> per-block exp bias path.
    biases: (qkv, proj, mlp1, mlp2) nonzero-bias flags."""
    import concourse.mybir as mybir
    import concourse.tile as tile
    from concourse import bacc
    from concourse.masks import make_identity

    f32 = mybir.dt.float32
    bf16 = mybir.dt.bfloat16
    dt_act = bf16 if dt_name == "bf16" else f32
    AF = mybir.ActivationFunctionType
    ALU = mybir.AluOpType
    X = mybir.AxisListType.X

    CH = SS // W          # seq rows per core
    NB = SS // 128        # 128-blocks along full sequence
    B = CH // 128         # 128-blocks per chunk
    NQ = SS // QC         # 512-wide query chunks
    use_qkv_b, use_proj_b, use_mlp1_b, use_mlp2_b = biases

    nc = bacc.Bacc(None)

    hid = nc.dram_tensor("hidden", [CH, E], f32, kind="ExternalInput")
    qkv_w = nc.dram_tensor("qkv_w", [128, 3 * 8 * 128], bf16, kind="ExternalInput")
    qkv_b = nc.dram_tensor("qkv_b", [1, 3 * 128], f32, kind="ExternalInput")
    proj_w = nc.dram_tensor("proj_w", [128, 8 * 8 * 128], bf16, kind="ExternalInput")
    proj_b = nc.dram_tensor("proj_b", [1, 8 * 128], f32, kind="ExternalInput")
    w1 = nc.dram_tensor("w1", [32 * 128, 8 * 128], bf16, kind="ExternalInput")
    b1 = nc.dram_tensor("b1", [1, 32 * 128], f32, kind="ExternalInput")
    w2 = nc.dram_tensor("w2", [8 * 128, 32 * 128], bf16, kind="ExternalInput")
    b2 = nc.dram_tensor("b2", [1, 8 * 128], f32, kind="ExternalInput")
    mask_b = nc.dram_tensor("mask_bias", [128, NB], f32, kind="ExternalInput")
    out = nc.dram_tensor("out", [CH, E], f32, kind="ExternalOutput")

    rg = [list(range(W))]

    with tile.TileContext(nc) as tc:
        with (
            tc.tile_pool(name="dram", bufs=1, space="DRAM") as dram,
            tc.tile_pool(name="const", bufs=1) as const,
            tc.tile_pool(name="persist", bufs=1) as persist,
            tc.tile_pool(name="work", bufs=2) as work,
            tc.tile_pool(name="xcc", bufs=2) as xccp,
            tc.tile_pool(name="exp", bufs=2) as exp_pool,
            tc.tile_pool(name="wstream", bufs=4) as wstream,
            tc.tile_pool(name="w2stream", bufs=2) as w2stream,
            tc.tile_pool(name="tp", bufs=2, space="PSUM") as tp_pool,
            tc.tile_pool(name="sc", bufs=2, space="PSUM") as sc_pool,
            tc.tile_pool(name="acc", bufs=2, space="PSUM") as acc_pool,
        ):
            # ----- constants -----
            ident = const.tile([128, 128], f32, tag="ident", name="ident")
            make_identity(nc, ident[:])
            ident_bf = const.tile([128, 128], bf16, tag="identbf", name="identbf")
            nc.vector.tensor_copy(ident_bf[:], ident[:])
            ident_a = ident_bf if dt_act != f32 else ident
            eps_sb = const.tile([128, 1], f32, tag="eps", name="eps")
            nc.vector.memset(eps_sb[:], 1e-5)
            ones_row = const.tile([1, QC], bf16, tag="ones_row", name="ones_row")
            nc.vector.memset(ones_row[:], 1.0)

            # absorb the first-collective entry barrier (~35us of cross-core
            # skew + ncfw warmup) behind LN1 with a tiny dummy AllGather, and
            # preload every ACT table set we use so no mid-kernel table-load
            # stall trips the PE HAM re-throttle.
            if not mock_cc:
                warm_sb = const.tile([1, 16], bf16, tag="warm", name="warm")
                nc.vector.memset(warm_sb[:], 0.0)
                warm_in = dram.tile([1, 16], bf16, name="warm_in")
                warm_out = dram.tile([W, 16], bf16, addr_space="Shared",
                                     name="warm_out")
                nc.sync.dma_start(out=warm_in[:, :], in_=warm_sb[:])
                nc.gpsimd.collective_compute(
                    "AllGather", ALU.bypass, replica_groups=rg,
                    ins=[warm_in.opt()], outs=[warm_out.opt()])
            dmy = const.tile([1, 2], f32, tag="dmy", name="dmy")
            nc.vector.memset(dmy[:], 1.0)
            for fn in (AF.Square, AF.Sqrt, AF.Exp, AF.Relu, AF.Identity):
                nc.scalar.activation(dmy[0:1, 1:2], dmy[0:1, 0:1], fn)

            warm_l = const.tile([1, 1], bf16, tag="warml", name="warml")
            nc.vector.memset(warm_l[:], 0.0)
            warm_r = const.tile([1, 512], bf16, tag="warmr", name="warmr")
            nc.vector.memset(warm_r[:], 0.0)
            warm_r32 = const.tile([1, 128], f32, tag="warmr32", name="warmr32")
            nc.vector.memset(warm_r32[:], 0.0)

            def pe_fill(n, pool=None):
                """n tiny matmuls (~213ns each) to keep the PE HAM
                un-throttled across a known engine-idle window."""
                p = pool or sc_pool
                tag = "sc" if p is sc_pool else "tp"
                for _ in range(n):
                    wt = p.tile([1, QC], f32, tag=tag, name="wfill")
                    nc.tensor.matmul(wt[:], lhsT=warm_l[:], rhs=warm_r[:],
                                     start=True, stop=True)

            def pe_fill_dep(dep_ap):
                """one filler matmul whose weights read `dep_ap`, so it runs
                right after the producer finishes — spreads keep-warm blips
                through otherwise PE-idle stretches."""
                wt = sc_pool.tile([1, QC], f32, tag="sc", name="wfill")
                nc.tensor.matmul(wt[:, 0:128], lhsT=dep_ap[0:1, 0:1],
                                 rhs=warm_r32[:], start=True, stop=True)

            mb_sb = None
            if masked:
                mb_sb = const.tile([128, NB], f32, tag="maskbias", name="maskbias")
                nc.sync.dma_start(out=mb_sb[:], in_=mask_b[:, :])

            def loadb(dram_t, n, name):
                t = const.tile([1, n], f32, tag=name, name=name)
                nc.sync.dma_start(out=t[:], in_=dram_t[:, :])
                return t

            qkv_b_sb = loadb(qkv_b, 3 * 128, "qkvb") if use_qkv_b else None
            proj_b_sb = loadb(proj_b, 8 * 128, "projb") if use_proj_b else None
            b1_sb = loadb(b1, 32 * 128, "b1") if use_mlp1_b else None
            b2_sb = loadb(b2, 8 * 128, "b2") if use_mlp2_b else None

            def dma(out_, in_):
                return nc.sync.dma_start(out=out_, in_=in_)

            # engine-alternating PSUM->SBUF evacuation
            ev_state = [0]

            def evac(out_, in_):
                ev_state[0] ^= 1
                if ev_state[0]:
                    nc.scalar.copy(out_, in_)
                else:
                    nc.vector.tensor_copy(out=out_, in_=in_)

            # ----- layer norm (plain: affine folded into next weights) -----
            def layer_norm_T(x, xnT, pfx):
                """x: [128, B*1024] rows tile; writes xnT [128, 8*CH] bf16
                (feature-block-major, seq-minor)."""
                ssum = work.tile([128, B], f32, tag="lnsum", name=f"{pfx}sum")
                ssq = work.tile([128, B], f32, tag="lnsq", name=f"{pfx}sq")
                for t in range(B):
                    xs = x[:, t * E:(t + 1) * E]
                    nc.vector.reduce_sum(out=ssum[:, t:t + 1], in_=xs, axis=X)
                    pe_fill_dep(ssum[:, t:t + 1])
                    scr = work.tile([128, E], f32, tag="lnscr", name="lnscr")
                    nc.scalar.activation(scr[:], xs, AF.Square,
                                         accum_out=ssq[:, t:t + 1])
                    pe_fill_dep(ssq[:, t:t + 1])
                st = work.tile([128, 4 * B], f32, tag="lnst", name=f"{pfx}st")
                mean, msq, var, nmr = (st[:, i * B:(i + 1) * B] for i in range(4))
                nc.vector.tensor_scalar_mul(mean, ssum[:], 1.0 / E)
                nc.vector.tensor_scalar_mul(msq, ssq[:], 1.0 / E)
                nc.vector.tensor_tensor(out=var, in0=mean, in1=mean, op=ALU.mult)
                nc.vector.tensor_tensor(out=var, in0=msq, in1=var, op=ALU.subtract)
                nc.scalar.activation(var, var, AF.Sqrt, bias=eps_sb[:], scale=1.0)
                nc.vector.reciprocal(out=var, in_=var)   # var now holds r (tiny)
                nc.vector.tensor_tensor(out=nmr, in0=mean, in1=var, op=ALU.mult)
                pe_fill_dep(var)
                nc.vector.tensor_scalar_mul(nmr, nmr, -1.0)
                pe_fill_dep(nmr)
                xnT_v = xnT[:].rearrange("p (m s) -> p m s", m=8)
                for t in range(B):
                    xn = work.tile([128, E], f32, tag="lnscr", name="lnxn")
                    nc.vector.tensor_scalar(out=xn[:], in0=x[:, t * E:(t + 1) * E],
                                            scalar1=var[:, t:t + 1],
                                            scalar2=nmr[:, t:t + 1],
                                            op0=ALU.mult, op1=ALU.add)
                    pe_fill_dep(xn[:])
                    for half in range(2):
                        tp = tp_pool.tile([128, 512], f32, tag="tp", name="tp")
                        for q in range(4):
                            m = half * 4 + q
                            nc.tensor.transpose(tp[:, q * 128:(q + 1) * 128],
                                                xn[:, m * 128:(m + 1) * 128],
                                                ident[:])
                        evac(xnT_v[:, half * 4:(half + 1) * 4,
                                   t * 128:(t + 1) * 128], tp[:])

            # ----- stage 1: load rows + LN1 -----
            x_rows = persist.tile([128, B * E], f32, tag="xrows", name="xrows")
            for t in range(B):
                dma(x_rows[:, t * E:(t + 1) * E], hid[t * 128:(t + 1) * 128, :])
            xnT = persist.tile([128, 8 * CH], bf16, tag="xnT", name="xnT")
            layer_norm_T(x_rows, xnT, "ln1")

            # ----- stage 2: AllGather normalized-transposed chunks (bf16) -----
            ag_in = dram.tile([E, CH], bf16, name="ag_in")
            ag_out = dram.tile([W * E, CH], bf16, addr_space="Shared",
                               name="ag_out")
            for m in range(8):
                dma(ag_in[m * 128:(m + 1) * 128, :], xnT[:, m * CH:(m + 1) * CH])
            if mock_cc:
                nc.sync.dma_start(out=ag_out[0:E, :], in_=ag_in[:, :])
            else:
                nc.gpsimd.collective_compute(
                    "AllGather", ALU.bypass, replica_groups=rg,
                    ins=[ag_in.opt()], outs=[ag_out.opt()])

            # ----- stage 3: qkv for this core's 2 heads -----
            wqkv = persist.tile([128, 3 * 8 * 128], bf16, tag="wqkv", name="wqkv")
            dma(wqkv[:], qkv_w[:, :])

            qT = persist.tile([128, SS], dt_act, tag="qT", name="qT")
            kT = persist.tile([128, SS], dt_act, tag="kT", name="kT")
            vT = persist.tile([128, SS], dt_act, tag="vT", name="vT")
            qkvT = [qT, kT, vT]
            for cc in range(NQ):
                g0, g1 = cc * QC, (cc + 1) * QC
                xg = [xccp.tile([128, QC], bf16, tag=f"xcc{k}", name=f"xcc{k}")
                      for k in range(8)]
                for k in range(8):
                    j0, j1 = g0 // CH, (g1 - 1) // CH
                    for j in range(j0, j1 + 1):
                        a, b_ = max(g0, j * CH), min(g1, (j + 1) * CH)
                        dma(xg[k][:, a - g0:b_ - g0],
                            ag_out[j * E + k * 128:j * E + (k + 1) * 128,
                                   a - j * CH:b_ - j * CH])
                for c in range(3):
                    ps = acc_pool.tile([128, QC], f32, tag="acc", name="acc")
                    for k in range(8):
                        nc.tensor.matmul(
                            ps[:],
                            lhsT=wqkv[:, (c * 8 + k) * 128:(c * 8 + k + 1) * 128],
                            rhs=xg[k][:],
                            start=(k == 0),
                            stop=(k == 7 and not use_qkv_b))
                    if use_qkv_b:
                        nc.tensor.matmul(
                            ps[:], lhsT=qkv_b_sb[:, c * 128:(c + 1) * 128],
                            rhs=ones_row[:], start=False, stop=True)
                    evac(qkvT[c][:, g0:g1], ps[:])

            # ----- stage 4: V transposed + ones-augmented column -----
            v_aug = [persist.tile([128, NB * 65], dt_act, tag=f"vaug{h}",
                                  name=f"vaug{h}") for h in range(2)]
            for h in range(2):
                vv = v_aug[h][:].rearrange("p (n c) -> p n c", c=65)
                nc.vector.memset(vv[:, :, 64:65], 1.0)
            for g in range(NB // 4):
                tpv = tp_pool.tile([128, 512], dt_act, tag="tp", name="tpv")
                for q in range(4):
                    tb = g * 4 + q
                    nc.tensor.transpose(tpv[:, q * 128:(q + 1) * 128],
                                        vT[:, tb * 128:(tb + 1) * 128],
                                        ident_a[:])
                tps = tpv[:].rearrange("p (n c) -> p n c", c=128)
                for h in range(2):
                    vv = v_aug[h][:].rearrange("p (n c) -> p n c", c=65)
                    nc.vector.tensor_copy(
                        out=vv[:, g * 4:(g + 1) * 4, 0:64],
                        in_=tps[:, :, 64 * h:64 * h + 64])

            wproj = persist.tile([128, 8 * 8 * 128], bf16, tag="wproj",
                                 name="wproj")
            dma(wproj[:], proj_w[:, :])

            # ----- stage 5+6: attention, head-outer; per-head AllToAll -----
            a2a_in = [dram.tile([W * 64, CH], bf16, name=f"a2a_in{h}")
                      for h in range(2)]
            a2a_out = [dram.tile([W * 64, CH], bf16, name=f"a2a_out{h}")
                       for h in range(2)]
            for h in range(2):
                e0 = 64 * h
                for qc in range(NQ):
                    n_t = (QC // 128) * (qc + 1)
                    av = acc_pool.tile([65, QC], f32, tag="acc", name="avacc")
                    for tp2 in range(n_t // 2):
                        tb0 = tp2 * 2
                        sc = sc_pool.tile([128, 2 * QC], f32, tag="sc", name="sc")
                        for u in range(2):
                            tb = tb0 + u
                            nc.tensor.matmul(
                                sc[:, u * QC:(u + 1) * QC],
                                lhsT=kT[e0:e0 + 64, tb * 128:(tb + 1) * 128],
                                rhs=qT[e0:e0 + 64, qc * QC:(qc + 1) * QC],
                                start=True, stop=True)
                        ex = exp_pool.tile([128, 2 * QC], dt_act, tag="ex",
                                           name="ex")
                        if masked:
                            for u in range(2):
                                tb = tb0 + u
                                nc.scalar.activation(
                                    ex[:, u * QC:(u + 1) * QC],
                                    sc[:, u * QC:(u + 1) * QC], AF.Exp,
                                    bias=mb_sb[:, tb:tb + 1], scale=1.0)
                        else:
                            nc.scalar.activation(ex[:], sc[:], AF.Exp)
                        p0 = tb0 - (QC // 128) * qc
                        if p0 >= 0:
                            # zero future (t > s) lanes of the 2 diag blocks
                            nc.gpsimd.affine_select(
                                out=ex[:], in_=ex[:],
                                compare_op=ALU.is_ge,
                                fill=0.0,
                                base=-128 * p0,
                                channel_multiplier=-1,
                                pattern=[[-128, 2], [1, QC]],
                            )
                        for u in range(2):
                            tb = tb0 + u
                            nc.tensor.matmul(
                                av[:],
                                lhsT=v_aug[h][:, tb * 65:(tb + 1) * 65],
                                rhs=ex[:, u * QC:(u + 1) * QC],
                                start=(tb == 0), stop=(tb == n_t - 1))
                        pe_fill(1, pool=tp_pool)
                    # evacuate av immediately so the PSUM accumulator slot
                    # frees before the (slow, 1-lane) reciprocal chain runs
                    avh = work.tile([64, QC], bf16, tag="avh", name="avh")
                    nc.vector.tensor_copy(out=avh[:], in_=av[0:64, :])
                    dn = work.tile([1, QC], f32, tag="dn", name="dn")
                    nc.scalar.copy(dn[:], av[64:65, :])
                    recip = work.tile([1, QC], f32, tag="recip", name="recip")
                    nc.vector.reciprocal(out=recip[:], in_=dn[:])
                    bc = work.tile([64, QC], f32, tag="bc", name="bc")
                    nc.gpsimd.partition_broadcast(bc[:], recip[:])
                    avs = work.tile([64, QC], bf16, tag="avsc", name="avsc")
                    nc.vector.tensor_tensor(out=avs[:], in0=avh[:],
                                            in1=bc[:], op=ALU.mult)
                    g0, g1 = qc * QC, (qc + 1) * QC
                    for j in range(g0 // CH, (g1 - 1) // CH + 1):
                        a, b_ = max(g0, j * CH), min(g1, (j + 1) * CH)
                        dma(a2a_in[h][j * 64:(j + 1) * 64, a - j * CH:b_ - j * CH],
                            avs[:, a - g0:b_ - g0])
                if mock_cc:
                    nc.sync.dma_start(out=a2a_out[h][:, :], in_=a2a_in[h][:, :])
                else:
                    nc.gpsimd.collective_compute(
                        "AllToAll", ALU.bypass, replica_groups=rg,
                        ins=[a2a_in[h].opt()], outs=[a2a_out[h].opt()])
                if h == 0:
                    pe_fill(12)

            # ----- stage 7a: proj over head-0 rows (K=64 upper half of every
            # k block) — placed after both heads in the PE stream so it fills
            # the AllToAll[h1] wait (its A2A[h0] input is long since done) -----
            aT0 = [persist.tile([64, CH], bf16, tag=f"aT0_{k}",
                                name=f"aT0_{k}") for k in range(8)]
            for k in range(8):
                dma(aT0[k][:], a2a_out[0][k * 64:(k + 1) * 64, :])
            partA = persist.tile([128, 8 * CH], bf16, tag="partA",
                                 name="partA")
            for m in range(8):
                psA = acc_pool.tile([128, QC], f32, tag="acc", name="acc")
                for k in range(8):
                    nc.tensor.matmul(
                        psA[:, 0:CH],
                        lhsT=wproj[0:64,
                                   (m * 8 + k) * 128:(m * 8 + k + 1) * 128],
                        rhs=aT0[k][:],
                        start=(k == 0), stop=(k == 7))
                evac(partA[:, m * CH:(m + 1) * CH], psA[:, 0:CH])
            pe_fill(40)

            # ----- stage 7b: proj (head-1 rows) + partA + residual -----
            aT1 = [persist.tile([128, CH], bf16, tag=f"aT1_{k}", name=f"aT1_{k}")
                   for k in range(8)]
            for k in range(8):
                dma(aT1[k][64:128, :], a2a_out[1][k * 64:(k + 1) * 64, :])
            res1 = persist.tile([128, B * E], f32, tag="res1", name="res1")
            res1_v = res1[:].rearrange("p (t e) -> p t e", e=E)
            xr_v = x_rows[:].rearrange("p (t e) -> p t e", e=E)
            for m in range(8):
                ps = acc_pool.tile([128, QC], f32, tag="acc", name="acc")
                for k in range(8):
                    nc.tensor.matmul(
                        ps[:, 0:CH],
                        lhsT=wproj[64:128,
                                   (m * 8 + k) * 128:(m * 8 + k + 1) * 128],
                        rhs=aT1[k][64:128, :],
                        start=(k == 0), stop=(k == 7 and not use_proj_b))
                if use_proj_b:
                    nc.tensor.matmul(
                        ps[:, 0:CH], lhsT=proj_b_sb[:, m * 128:(m + 1) * 128],
                        rhs=ones_row[:, 0:CH], start=False, stop=True)
                pTm = work.tile([128, CH], f32, tag="pTm", name="pTm")
                nc.vector.tensor_tensor(out=pTm[:], in0=ps[:, 0:CH],
                                        in1=partA[:, m * CH:(m + 1) * CH],
                                        op=ALU.add)
                tpp = tp_pool.tile([128, 512], f32, tag="tp", name="tpp")
                for t in range(B):
                    nc.tensor.transpose(tpp[:, t * 128:(t + 1) * 128],
                                        pTm[:, t * 128:(t + 1) * 128],
                                        ident[:])
                tps = tpp[:, 0:B * 128].rearrange("p (t e) -> p t e", e=128)
                nc.vector.tensor_tensor(
                    out=res1_v[:, :, m * 128:(m + 1) * 128],
                    in0=tps, in1=xr_v[:, :, m * 128:(m + 1) * 128],
                    op=ALU.add)

            # ----- stage 8: LN2 (ln2 affine folded into w1) -----
            l2T = persist.tile([128, 8 * CH], bf16, tag="l2T", name="l2T")
            layer_norm_T(res1, l2T, "ln2")

            # ----- stage 9: MLP (full, on this core's seq chunk; bf16) -----
            h1T = [persist.tile([128, CH], bf16, tag=f"h1T{m}", name=f"h1T{m}")
                   for m in range(32)]
            for m in range(32):
                w1m = wstream.tile([128, 8 * 128], bf16, tag="w1m", name="w1m")
                dma(w1m[:], w1[m * 128:(m + 1) * 128, :])
                ps = acc_pool.tile([128, QC], f32, tag="acc", name="acc")
                for k in range(8):
                    nc.tensor.matmul(
                        ps[:, 0:CH], lhsT=w1m[:, k * 128:(k + 1) * 128],
                        rhs=l2T[:, k * CH:(k + 1) * CH],
                        start=(k == 0), stop=(k == 7 and not use_mlp1_b))
                if use_mlp1_b:
                    nc.tensor.matmul(
                        ps[:, 0:CH], lhsT=b1_sb[:, m * 128:(m + 1) * 128],
                        rhs=ones_row[:, 0:CH], start=False, stop=True)
                if m % 2 == 0:
                    nc.scalar.activation(h1T[m][:], ps[:, 0:CH], AF.Relu)
                else:
                    nc.vector.tensor_scalar(out=h1T[m][:], in0=ps[:, 0:CH],
                                            scalar1=0.0, scalar2=None,
                                            op0=ALU.max)

            oT = [persist.tile([128, CH], bf16, tag=f"oT{m}", name=f"oT{m}")
                  for m in range(8)]
            for m in range(8):
                ps = acc_pool.tile([128, QC], f32, tag="acc", name="acc")
                for half in range(2):
                    w2m = w2stream.tile([128, 16 * 128], bf16, tag="w2m",
                                        name="w2m")
                    dma(w2m[:], w2[m * 128:(m + 1) * 128,
                                   half * 16 * 128:(half + 1) * 16 * 128])
                    for kk in range(16):
                        k = half * 16 + kk
                        nc.tensor.matmul(
                            ps[:, 0:CH], lhsT=w2m[:, kk * 128:(kk + 1) * 128],
                            rhs=h1T[k][:],
                            start=(k == 0), stop=(k == 31 and not use_mlp2_b))
                if use_mlp2_b:
                    nc.tensor.matmul(
                        ps[:, 0:CH], lhsT=b2_sb[:, m * 128:(m + 1) * 128],
                        rhs=ones_row[:, 0:CH], start=False, stop=True)
                evac(oT[m][:], ps[:, 0:CH])

            # ----- stage 10: transpose back + final residual + out -----
            for t in range(B):
                tpo = tp_pool.tile([128, 8 * 128], bf16, tag="tp", name="tpo")
                for m in range(8):
                    nc.tensor.transpose(tpo[:, m * 128:(m + 1) * 128],
                                        oT[m][:, t * 128:(t + 1) * 128],
                                        ident_bf[:])
                orow = work.tile([128, E], f32, tag="orow", name="orow")
                nc.vector.tensor_tensor(
                    out=orow[:], in0=tpo[:],
                    in1=res1[:, t * E:(t + 1) * E], op=ALU.add)
                dma(out[t * 128:(t + 1) * 128, :], orow[:])

    return nc


def _prepare_in_maps(inputs, SS: int):
    """Host-side prep: fold LN affines into the following matmuls (exact),
    slice per core, prescale q by 1/8, pre-tile all weights contiguously,
    bf16-cast matmul weights."""
    import ml_dtypes

    bf16 = ml_dtypes.bfloat16
    CH = SS // W
    NB = SS // 128
    hid = np.ascontiguousarray(
        np.asarray(inputs["hidden_states"], np.float32)[0, :SS])

    ln1_w = np.asarray(inputs["ln1_w"], np.float32)
    ln1_b = np.asarray(inputs["ln1_b"], np.float32)
    ln2_w = np.asarray(inputs["ln2_w"], np.float32)
    ln2_b = np.asarray(inputs["ln2_b"], np.float32)

    # fold LN1 into qkv conv
    attn_w = ln1_w[:, None] * np.asarray(inputs["attn_w"], np.float32)
    attn_b = (np.asarray(inputs["attn_b"], np.float32)
              + ln1_b @ np.asarray(inputs["attn_w"], np.float32))
    attn_w[:, :E] *= 0.125
    attn_b[:E] *= 0.125

    # fold LN2 into mlp w1
    w1 = ln2_w[:, None] * np.asarray(inputs["mlp_w1"], np.float32)
    b1 = (np.asarray(inputs["mlp_b1"], np.float32)
          + ln2_b @ np.asarray(inputs["mlp_w1"], np.float32))

    proj_w = np.asarray(inputs["proj_w"], np.float32)
    proj_b = np.asarray(inputs["proj_b"], np.float32)
    w2 = np.asarray(inputs["mlp_w2"], np.float32)
    b2 = np.asarray(inputs["mlp_b2"], np.float32)

    mask = np.asarray(inputs["mask"])[0, 0, 0, :SS]
    masked = not bool(mask.all())
    mask_bias = np.where(mask, 0.0, MASK).astype(np.float32)

    # X[k*128+p, m*128+f] -> [(m p), (k f)]  (k-major inside a row-block)
    def tile_mk(x, km, mm_):
        return np.ascontiguousarray(
            x.reshape(km, 128, mm_, 128).transpose(2, 1, 0, 3)
            .reshape(mm_ * 128, km * 128))

    # proj tiled as [128, m, k, 128] single row-block for one contiguous DMA
    proj_t = (proj_w.reshape(8, 128, 8, 128).transpose(1, 2, 0, 3)
              .reshape(128, 8 * 8 * 128))

    biases = (bool(np.any(attn_b)), bool(np.any(proj_b)),
              bool(np.any(b1)), bool(np.any(b2)))

    common = {
        "proj_w": np.ascontiguousarray(proj_t).astype(bf16),
        "proj_b": np.ascontiguousarray(proj_b.reshape(1, -1)),
        "w1": tile_mk(w1, 8, 32).astype(bf16),
        "b1": np.ascontiguousarray(b1.reshape(1, -1)),
        "w2": tile_mk(w2, 32, 8).astype(bf16),
        "b2": np.ascontiguousarray(b2.reshape(1, -1)),
        "mask_bias": np.ascontiguousarray(mask_bias.reshape(NB, 128).T),
    }
    in_maps = []
    for i in range(W):
        wq = np.empty((128, 3, 8, 128), np.float32)
        bq = np.empty((3, 128), np.float32)
        for c in range(3):
            cols = slice(c * E + 128 * i, c * E + 128 * i + 128)
            wq[:, c] = attn_w[:, cols].reshape(8, 128, 128).transpose(1, 0, 2)
            bq[c] = attn_b[cols]
        in_maps.append({
            "hidden": np.ascontiguousarray(hid[i * CH:(i + 1) * CH]),
            "qkv_w": np.ascontiguousarray(wq.reshape(128, -1)).astype(bf16),
            "qkv_b": np.ascontiguousarray(bq.reshape(1, -1)),
            **common,
        })
    return in_maps, masked, biases


def _run(inputs, SS, dt_name, **kw):
    from concourse.bass_utils import run_bass_kernel_spmd

    in_maps, masked, biases = _prepare_in_maps(inputs, SS)
    key = (SS, dt_name, masked, biases)
    if key not in _CACHE:
        nc = _build(SS, dt_name, masked=masked, biases=biases)
        nc.finalize()
        _CACHE[key] = nc
    nc = _CACHE[key]
    res = run_bass_kernel_spmd(nc, in_maps, core_ids=list(range(W)), **kw)
    full = np.concatenate([r["out"] for r in res.results], axis=0)
    return full[None].astype(np.float32), res


def kernel(**inputs) -> np.ndarray:
    out, _ = _run(inputs, 3072, "bf16")
    return out
